# revision 49
# baseline (speedup 1.0000x reference)
"""Trainium2 Bass kernel for nn_Aggregation_74904229642960 (gnn_message_passing).

The reference computes, with tgt = edge_index[1]:

    sm  = segment_softmax(x, tgt, N)   # per-(target node, feature) softmax over edges
    out = segment_sum(sm, tgt, N)      # [N, d]

The final segment_sum contracts exactly the segments the softmax normalized
over, and softmax weights sum to 1 over their own segment.  Hence, exactly
(independent of x, which only shifts/scales terms that cancel):

    out[n, f] = 1.0  if node n has >= 1 incoming edge, else 0.0

The kernel therefore reads only edge_index[1]: it computes the in-degree
histogram (bincount over the 10000 nodes) on device and emits 1.0 rows for
nodes with nonzero degree.

Sharding (8 NeuronCores): edges are split E/8 per core, each core builds a
partial per-node 0/1 indicator, the partials are combined with TWO
ReduceScatter(add) collectives split by edge subsets -- the first (tiles
[0, SPLIT)) runs while the rest of the histogram computes, absorbing the
collective path's cold-start cost (~25-40us wall in this environment),
so only the second, warmer collective (~14-20us) is exposed at the end.
Each core keeps the 10 hi-rows of its own 1280-node slice and writes its
1/8 of the [N, d] output; the host concatenates.  (AllToAll + local
selector-matmul sum was measured slower: ~33us on this topology.)

Per-core bincount (80000 edges = exactly 625 tiles of 128), n = hi*128+lo:
  for each tile of 128 edges (one edge per SBUF partition):
      A[e, :] = onehot80(hi_e)    # bf16 is_equal against an iota table
      B[e, :] = onehot128(lo_e)
      counts[hi, lo] += A^T @ B   # PE matmul, fp32 PSUM accumulation

Performance notes (measured on trn2):
  * Both matmul operands are built m-inner so they are contiguous (a strided
    moving operand streams ~4x slower, strided LdWeights ~5x slower).
  * The DVE packed 2x compare mode needs step-1 innermost on EVERY operand,
    which a digit-broadcast input violates.  The otherwise-idle Scalar
    engine pre-replicates both digit streams so every is_equal runs at 2x.
    Each digit's bf16 bit pattern is pre-duplicated BY THE HOST into both
    halves of an int32, so ACT replicates at fp32 width (half the elements)
    and the result is bitcast back to bf16 pairs (exact for digits 0..127).
    Host pre-packing removes ~6.5us of digit-extraction work the DVE used
    to do on device (the DVE is the critical engine at ~104 cyc/tile).
    Both replications land in ONE interleaved [p, j, (80 hi | 128 lo)]
    tile compared against a combined 208-wide iota table, so the DVE runs
    a single is_equal per group (measured 109.4ns/tile, within 1-2% of
    the 2x-mode floor); per-tile matmul operand slices stay contiguous.
  * The edge stream is uploaded in 3 chunks so group 0's replicate/compare
    starts as soon as ~45KB have landed instead of after the full 640KB.
  * The per-core histogram is clamped to a 0/1 indicator before each
    collective, so the ReduceScatter payload is bf16 and the summed
    partials (<= 8 per collective, <= 16 combined) are exact.
  * After the second collective only chunk B's transpose + a [128, 10]
    add + one broadcast is_gt + the output DMA remain on the critical
    path; chunk A is transposed while the second collective is in flight.
"""

import os

import numpy as np

import concourse.bass as bass
import concourse.mybir as mybir
import concourse.tile as tile
from concourse.bass_utils import run_bass_kernel_spmd

N_NODES = 10000
N_EDGES = 640000
D_FEAT = 128
N_CORES = 8

P = 128               # SBUF partitions / edges per tile
HI = 80               # hi-digit one-hot width (hi = n >> 7 in [0, 80))
LO = 128              # lo-digit one-hot width (lo = n & 127)
NODES_PAD = HI * LO   # 10240 >= N_NODES
ROWS_PER_CORE = NODES_PAD // N_CORES      # 1280 output rows per core
OUT_TILES = ROWS_PER_CORE // P            # 10 output tiles of 128 nodes

E_LOC = N_EDGES // N_CORES                # 80000 edges per core
NT = E_LOC // P                           # 625 tiles, exact (no padding)

# group sizes (tiles per ACT-replicate/DVE-compare instruction pair); small
# leading groups shorten pipeline-fill, bulk 64 amortizes the ~300ns
# per-group DVE instruction overhead, small trailing groups shorten the
# final matmul tail.
GROUPS = [8, 12, 24, 44, 44, 44] + [64] * 6 + [33, 20, 12]
assert sum(GROUPS) == NT
# input-chunk boundaries (tiles) -- each chunk is a separate DMA so early
# groups are not gated on the full edge upload; groups never span chunks.
CHUNKS = [0, 44, 176, NT]
N_CHUNKS = len(CHUNKS) - 1

f32 = mybir.dt.float32
bf16 = mybir.dt.bfloat16
i32 = mybir.dt.int32

LAST_RESULTS = None


def _ensure_ntff_hook():
    """Install the axon NTFF-profile hook if the container's antenv stub
    lacks it (profiling-only; kernel correctness does not depend on this)."""
    import sys
    import types

    try:
        from antenv.axon_hooks import get_axon_ntff_profile_hook  # noqa: F401

        return
    except ImportError:
        pass
    m = types.ModuleType("antenv.axon_hooks")
    m._hook = None
    m.set_axon_ntff_profile_hook = lambda h: setattr(m, "_hook", h)
    m.get_axon_ntff_profile_hook = lambda: m._hook
    import antenv

    sys.modules["antenv.axon_hooks"] = m
    antenv.axon_hooks = m
    try:
        from trn_agent_boot.trn_boot import _ntff_profile_via_ctypes

        hook = _ntff_profile_via_ctypes("/opt/axon/libaxon_pjrt.so")
        if hook is not None:
            m._hook = hook
    except Exception as e:  # profiling is best-effort
        print("ntff hook install failed:", e)


_ENGINE_SEM_PREFIX = {
    mybir.EngineType.PE: "PE_",
    mybir.EngineType.DVE: "DVE_",
    mybir.EngineType.Activation: "ACT_",
    mybir.EngineType.Pool: "POOL_",
    mybir.EngineType.SP: "SP_",
}


def _legalize_waits(nc: bass.Bass) -> None:
    """Walrus codegen allows a single sync-wait slot per ISA instruction;
    Tile can emit several.  Two-step legalization:

    1. Drop waits on the instruction's *own* engine completion semaphore when
       other waits are present (engines execute serially, so Tile's same-
       engine WAW guard is implied by program order).
    2. Hoist any remaining extra waits onto standalone EventSemaphore
       instructions inserted just before the owner on the same engine.
    """
    n_split = 0
    for f in nc.m.functions:
        for bb in f.blocks:
            new_insts = []
            for ins in bb.instructions:
                si = getattr(ins, "sync_info", None)
                if si is None or len(si.on_wait) < 2:
                    new_insts.append(ins)
                    continue
                waits = list(si.on_wait)
                prefix = _ENGINE_SEM_PREFIX.get(ins.engine)
                if prefix is not None:
                    kept = [w for w in waits if not (w.ant_name or "").startswith(prefix)]
                    if kept:
                        waits = kept
                for w in waits[:-1]:
                    ev = mybir.InstEventSemaphore(
                        name=f"W-split-{n_split}", ins=[], outs=[]
                    )
                    n_split += 1
                    ev.engine = ins.engine
                    ev.sync_info = mybir.SyncInfo(
                        on_wait=[w],
                        on_update=[
                            mybir.SyncUpdate(
                                sync_type="semaphore",
                                id=w.id,
                                ant_name=w.ant_name,
                                update_mode="sem-add-imm",
                                update_value=0,
                            )
                        ],
                    )
                    new_insts.append(ev)
                ins.sync_info = mybir.SyncInfo(
                    on_wait=[waits[-1]], on_update=list(si.on_update)
                )
                new_insts.append(ins)
            bb.instructions[:] = new_insts


def build_nc(n_cores: int = N_CORES) -> bass.Bass:
    """Build the SPMD Bass program (one NEFF, run on all cores)."""
    nc = bass.Bass()

    # pk[p, 2j] / pk[p, 2j+1]: hi/lo digit of local edge j*128+p, as the
    # digit's bf16 bit pattern duplicated into both halves of an int32
    # (host-precomputed).  consts = [iota_hi | iota_lo | ident].
    n_chunk_cols = [2 * (CHUNKS[i + 1] - CHUNKS[i]) for i in range(N_CHUNKS)]
    pk_in = [
        nc.dram_tensor(f"pk{i}", [P, n_chunk_cols[i]], i32, kind="ExternalInput")
        for i in range(N_CHUNKS)
    ]
    consts_in = nc.dram_tensor(
        "consts", [P, HI + LO + P], bf16, kind="ExternalInput"
    )
    out_ext = nc.dram_tensor("out", [ROWS_PER_CORE, D_FEAT], f32, kind="ExternalOutput")

    with tile.TileContext(nc, num_cores=n_cores) as tc:
        with (
            tc.tile_pool(name="sbuf", bufs=1) as sb,
            tc.tile_pool(name="onehot", bufs=3) as oh,
            tc.tile_pool(name="outp", bufs=1) as op_pool,
            tc.tile_pool(name="psum", bufs=1, space="PSUM") as ps,
            tc.tile_pool(name="psum2", bufs=2, space="PSUM") as ps2,
            tc.tile_pool(name="dram", bufs=1, space="DRAM") as dram,
        ):
            # --- constant tables + chunked edge-digit upload ----------------
            # pk0 first: group 0's ACT replicate only needs pk0, so it gates
            # the whole pipeline start.
            pk_sb = []
            for i in range(N_CHUNKS):
                pk_sb.append(
                    sb.tile([P, n_chunk_cols[i]], i32, tag=f"pk{i}", name=f"pk{i}")
                )
            nc.sync.dma_start(out=pk_sb[0][:], in_=pk_in[0][:])
            consts = sb.tile([P, HI + LO + P], bf16)
            nc.sync.dma_start(out=consts[:], in_=consts_in[:])
            for i in range(1, N_CHUNKS):
                nc.sync.dma_start(out=pk_sb[i][:], in_=pk_in[i][:])
            iota_hl = consts[:][:, 0 : HI + LO]   # [iota_hi | iota_lo]
            ident = consts[:][:, HI + LO : HI + LO + P]

            # --- one-hots + matmul accumulation -----------------------------
            # two accumulators split by edge subsets: the first half's
            # partial is reduce-scattered WHILE the second half computes,
            # hiding most of one collective's ~25-35us wall latency.
            SPLIT = 88  # tiles in part A (a group boundary); RS#1 pays the
            # cold collective cost (~25-50us wall in this environment), so
            # launch it as early as possible — its input is ready ~25us in,
            # leaving ~60us of compute to hide under before RS#2 needs the
            # serial collective engine.
            counts_psA = ps.tile([HI, LO], f32, space="PSUM")
            counts_psB = ps.tile([HI, LO], f32, space="PSUM")

            def emit_group(j0, gsz):
                # locate the chunk holding tiles [j0, j0+gsz)
                ci = max(i for i in range(N_CHUNKS) if CHUNKS[i] <= j0)
                assert j0 + gsz <= CHUNKS[ci + 1]
                # packed fp32 view of this group's digit pairs: [p, j, 2]
                pk_f = (
                    pk_sb[ci][:]
                    .bitcast(f32)
                    .rearrange("p (j t) -> p j t", t=2)[
                        :, j0 - CHUNKS[ci] : j0 - CHUNKS[ci] + gsz, :
                    ]
                )
                # ACT replicates both digit streams at fp32 width (bf16
                # pairs) into ONE interleaved tile [p, j, (80 hi | 128 lo)]
                # so the DVE runs a SINGLE 2x-mode is_equal per group
                # (halves DVE instruction overhead + inter-op drains).
                HL = HI + LO
                rep = oh.tile([P, gsz * HL], bf16, tag="rep")
                rep_f = rep[:].bitcast(f32).rearrange(
                    "p (j m) -> p j m", m=HL // 2
                )
                nc.scalar.activation(
                    out=rep_f[:, :, 0 : HI // 2],
                    in_=pk_f[:, :, 0:1].to_broadcast([P, gsz, HI // 2]),
                    func=mybir.ActivationFunctionType.Copy,
                )
                nc.scalar.activation(
                    out=rep_f[:, :, HI // 2 :],
                    in_=pk_f[:, :, 1:2].to_broadcast([P, gsz, LO // 2]),
                    func=mybir.ActivationFunctionType.Copy,
                )
                # DVE 2x-mode one-hot compare (all operands step-1 innermost)
                onehot = oh.tile([P, gsz * HL], bf16, tag="onehot")
                nc.vector.tensor_tensor(
                    out=onehot[:].rearrange("p (j m) -> p j m", m=HL),
                    in0=rep[:].rearrange("p (j m) -> p j m", m=HL),
                    in1=iota_hl[:, None, :].to_broadcast([P, gsz, HL]),
                    op=mybir.AluOpType.is_equal,
                )
                for j in range(gsz):
                    jj = j0 + j
                    part = counts_psA if jj < SPLIT else counts_psB
                    nc.tensor.matmul(
                        out=part[:],
                        lhsT=onehot[:][:, j * HL : j * HL + HI],
                        rhs=onehot[:][:, j * HL + HI : (j + 1) * HL],
                        start=(jj == 0 or jj == SPLIT),
                        stop=(jj == SPLIT - 1 or jj == NT - 1),
                    )

            def emit_rs_send(counts_ps, tag):
                # clamp partial histogram to 0/1 (bf16 exact), then
                # ReduceScatter(add): sums <= 8 exact; each core keeps the
                # 10 hi-rows of its own 1280-node slice.  (AllToAll + local
                # sum was tried: ~33us on this topology vs ~12-30us for RS.)
                counts_sb = sb.tile([HI, LO], bf16, name=f"counts_sb_{tag}")
                nc.vector.tensor_scalar(
                    out=counts_sb[:], in0=counts_ps[:], scalar1=0.0,
                    scalar2=None, op0=mybir.AluOpType.is_gt,
                )
                cc_in = dram.tile([HI, LO], bf16, name=f"cc_in_{tag}")
                cc_out = dram.tile(
                    [HI // n_cores, LO], bf16, name=f"cc_out_{tag}"
                )
                nc.sync.dma_start(out=cc_in[:], in_=counts_sb[:])
                nc.gpsimd.collective_compute(
                    "ReduceScatter",
                    mybir.AluOpType.add,
                    replica_groups=[list(range(n_cores))],
                    ins=[cc_in[:]],
                    outs=[cc_out[:]],
                )
                return cc_out

            def emit_rs_recv(cc_out, tag):
                chunk_sb = sb.tile([OUT_TILES, LO], bf16, name=f"chunk_{tag}")
                nc.sync.dma_start(out=chunk_sb[:], in_=cc_out[:])
                return chunk_sb

            j0 = 0
            cc_out_a = None
            for gi, gsz in enumerate(GROUPS):
                emit_group(j0, gsz)
                j0 += gsz
                if j0 == SPLIT:
                    cc_out_a = emit_rs_send(counts_psA, "a")
            cc_out_b = emit_rs_send(counts_psB, "b")
            # readback DMAs AFTER cc_in_b on the in-order Sync engine:
            # chunk_a's readback waits on RS#1 and must not block RS#2's
            # input upload.  The scheduler's cost model assumes collectives
            # are fast and would otherwise hoist these — force them late
            # with manual wait hints (pure scheduling priority, no delay).
            with tc.tile_wait_until(0.15):
                chunk_a = emit_rs_recv(cc_out_a, "a")
                # transpose chunk A as soon as RS#1 lands — free work during
                # the RS#2 wait; keeps only chunk B's transpose + combine on
                # the post-collective critical path.
                deg_ta_ps = ps2.tile([P, OUT_TILES], bf16, space="PSUM")
                nc.tensor.transpose(
                    out=deg_ta_ps[:], in_=chunk_a[:],
                    identity=ident[:OUT_TILES, :OUT_TILES],
                )
                deg_ta = sb.tile([P, OUT_TILES], bf16)
                nc.vector.tensor_copy(out=deg_ta[:], in_=deg_ta_ps[:])
                chunk_b = emit_rs_recv(cc_out_b, "b")

            with tc.tile_wait_until(0.2):
                deg_tb_ps = ps2.tile([P, OUT_TILES], bf16, space="PSUM")
                nc.tensor.transpose(
                    out=deg_tb_ps[:], in_=chunk_b[:],
                    identity=ident[:OUT_TILES, :OUT_TILES],
                )
                degsum = sb.tile([P, OUT_TILES], bf16)
                nc.vector.tensor_tensor(
                    out=degsum[:], in0=deg_ta[:], in1=deg_tb_ps[:],
                    op=mybir.AluOpType.add,
                )

                # --- emit output rows: 1.0 where deg > 0, one DVE op --------
                # (a two-half o_all + pipelined DMAs was measured: the
                # halves drop out of the fast DVE mode, netting a wash)
                o_all = op_pool.tile([P, OUT_TILES * D_FEAT], f32)
                nc.vector.tensor_scalar(
                    out=o_all[:].rearrange("p (k f) -> p k f", f=D_FEAT),
                    in0=degsum[:][:, :, None].to_broadcast(
                        [P, OUT_TILES, D_FEAT]
                    ),
                    scalar1=0.0,
                    scalar2=None,
                    op0=mybir.AluOpType.is_gt,
                )
                nc.sync.dma_start(
                    out=out_ext[:].rearrange("(k p) f -> p k f", p=P),
                    in_=o_all[:].rearrange("p (k f) -> p k f", f=D_FEAT),
                )

    _legalize_waits(nc)
    return nc


_NC_CACHE: dict = {}


def _host_pack(tgt: np.ndarray) -> list[np.ndarray]:
    """Per-core packed digit streams: [128, 2*NT] int32, col 2j = hi digit of
    tile j, col 2j+1 = lo digit, each as the digit's bf16 bit pattern
    duplicated into both int32 halves (exact for 0..127)."""
    packs = []
    for c in range(N_CORES):
        t = tgt[c * E_LOC : (c + 1) * E_LOC].reshape(NT, P).T  # [128, 625]
        hi = (t >> 7).astype(np.int64)
        lo = (t & 127).astype(np.int64)

        def pk(d):
            bits16 = (d.astype(np.float32).view(np.uint32) >> 16).astype(np.int64)
            return (bits16 | (bits16 << 16)).astype(np.uint32).view(np.int32)

        arr = np.empty((P, NT, 2), np.int32)
        arr[:, :, 0] = pk(hi)
        arr[:, :, 1] = pk(lo)
        packs.append(np.ascontiguousarray(arr.reshape(P, 2 * NT)))
    return packs


def kernel(**inputs: np.ndarray) -> np.ndarray:
    global LAST_RESULTS
    edge_index = np.asarray(inputs["edge_index"])
    assert edge_index.shape == (2, N_EDGES), edge_index.shape
    tgt = np.ascontiguousarray(edge_index[1].astype(np.int32))

    if "nc" not in _NC_CACHE:
        _NC_CACHE["nc"] = build_nc()
    nc = _NC_CACHE["nc"]

    import ml_dtypes

    iota_hi = np.broadcast_to(np.arange(HI, dtype=np.float32), (P, HI))
    iota_lo = np.broadcast_to(np.arange(LO, dtype=np.float32), (P, LO))
    ident = np.eye(P, dtype=np.float32)
    consts = np.ascontiguousarray(
        np.concatenate([iota_hi, iota_lo, ident], axis=1)
    ).astype(ml_dtypes.bfloat16)

    packs = _host_pack(tgt)
    in_maps = []
    for c in range(N_CORES):
        m = {"consts": consts}
        for i in range(N_CHUNKS):
            m[f"pk{i}"] = np.ascontiguousarray(
                packs[c][:, 2 * CHUNKS[i] : 2 * CHUNKS[i + 1]]
            )
        in_maps.append(m)

    trace = bool(int(os.environ.get("KERNEL_TRACE", "0")))
    if trace:
        _ensure_ntff_hook()
    trace_cores = [
        int(c) for c in os.environ.get("KERNEL_TRACE_CORES", "0").split(",")
    ]
    res = run_bass_kernel_spmd(
        nc,
        in_maps,
        core_ids=list(range(N_CORES)),
        trace=trace,
        trace_cores=trace_cores,
    )
    LAST_RESULTS = res

    out = np.concatenate([res.results[c]["out"] for c in range(N_CORES)], axis=0)
    return np.ascontiguousarray(out[:N_NODES]).astype(np.float32)


if __name__ == "__main__":
    # quick self-test with random inputs (no reference needed)
    rng = np.random.default_rng(0)
    ei = rng.integers(0, N_NODES, size=(2, N_EDGES)).astype(np.int32)
    x = rng.standard_normal((N_EDGES, D_FEAT)).astype(np.float32)
    out = kernel(source_node_representation_with_coefficient=x, edge_index=ei)
    deg = np.bincount(ei[1], minlength=N_NODES)
    exp = (deg > 0).astype(np.float32)[:, None] * np.ones((1, D_FEAT), np.float32)
    print("match:", np.array_equal(out, exp), "out mean:", out.mean())
